# revision 1
# baseline (speedup 1.0000x reference)
import numpy as np
import jax
import jax.numpy as jnp
from jax.sharding import Mesh, NamedSharding, PartitionSpec as P

# nn_Head_63359357550851: single-head causal attention
# x:[4,4096,1024] f32, Wq/Wk/Wv:[1024,64] f32 -> out:[4,4096,64] f32
# Sharding: sequence-parallel — T split 8 ways across the NeuronCores
# (B=4 doesn't divide 8; T=4096 does). Weights replicated; XLA SPMD
# inserts the K/V all-gather needed for the causal attention.
B, T, C, H = 4, 4096, 1024, 64

def _attn(x, Wq, Wk, Wv):
    q = x @ Wq                                   # [B,T,H]
    k = x @ Wk
    v = x @ Wv
    scale = jnp.float32(C) ** -0.5
    wei = jnp.einsum('bth,bsh->bts', q, k) * scale
    causal = jnp.arange(T)[:, None] >= jnp.arange(T)[None, :]
    wei = jnp.where(causal, wei, -jnp.inf)
    wei = jax.nn.softmax(wei, axis=-1)
    return jnp.einsum('bts,bsh->bth', wei, v)    # [B,T,H]

_compiled = None

def kernel(x, Wq, Wk, Wv):
    global _compiled
    if _compiled is None:
        devs = np.array(jax.devices()[:8])
        mesh = Mesh(devs, ('i',))
        xsh = NamedSharding(mesh, P(None, 'i', None))   # shard T
        wsh = NamedSharding(mesh, P())                  # replicate
        _compiled = jax.jit(_attn,
                            in_shardings=(xsh, wsh, wsh, wsh),
                            out_shardings=xsh)
    out = _compiled(jnp.asarray(x, jnp.float32),
                    jnp.asarray(Wq, jnp.float32),
                    jnp.asarray(Wk, jnp.float32),
                    jnp.asarray(Wv, jnp.float32))
    return np.asarray(jax.device_get(out), dtype=np.float32)



# revision 9
# speedup vs baseline: 7395.1440x; 7395.1440x over previous
"""nn_Head_63359357550851: single-head causal attention on 8 TRN2 NeuronCores.

x:[4,4096,1024] f32, Wq/Wk/Wv:[1024,64] f32 -> out:[4,4096,64] f32
  out = softmax(causal(x@Wq @ (x@Wk)^T / sqrt(1024))) @ (x@Wv)

Sharding: core c = (batch b=c//2, parity j=c%2). Each batch is split into 16
query chunks of 256; core (b,j) owns chunks {2k+j : k=0..7} (even/odd
interleave -> perfectly balanced causal work after padding each slot k to
2k+2 key chunks). K/V are projected shard-locally and exchanged within the
(b,0)/(b,1) pair via two pairwise AllGathers (one per half of the sequence)
so attention on early slots overlaps the gather of late keys.

Device kernel (SPMD, identical program on all 8 cores; only data differs):
  phase 1: QT/KT via packed [Wq|Wk] stationary matmuls (N=512), V in natural
           [s,h] layout via xt-stationary matmuls; bf16 in, f32 PSUM.
  phase 2: S^T-layout flash attention: S^T[s,q] = KT_blk^T @ QT, exp on ACT
           with the 1/sqrt(C) scale folded in (no max pass - scores are
           bounded: std ~0.25), causal masks built on-device from two static
           diagonal tiles + a per-core 4-scalar selector, AV accumulated as
           O^T = Vaug^T @ P^T with a ones-column in V producing the softmax
           denominator for free.
  epilogue: PE-transpose O^T -> O, multiply by reciprocal denominator, f32 out.
"""

import sys

import numpy as np

B, T, C, H = 4, 4096, 1024, 64
NCORES = 8
QC = 256  # queries per chunk
SLOTS = 8  # chunks per core
SHARD_T = SLOTS * QC  # 2048 rows per core

_BASS_PATHS = ["/opt/trn_rl_repo", "/root/.axon_site/_ro/trn_rl_repo"]

_cache = {}


def _ensure_import_path():
    for p in _BASS_PATHS:
        if p not in sys.path:
            sys.path.append(p)


def _gpos(sb):
    """Global s128-block index -> tile position in the gathered KV buffers."""
    g, e = sb // 2, sb % 2  # key chunk, sub-block
    half, p, l = g // 8, g % 2, (g // 2) % 4
    return half * 16 + p * 8 + l * 2 + e


def _build():
    _ensure_import_path()
    import concourse.bass as bass
    import concourse.mybir as mybir
    import concourse.tile as tile
    from concourse.vector_clock import ScopedClock

    f32 = mybir.dt.float32
    bf16 = mybir.dt.bfloat16
    mult = mybir.AluOpType.mult
    add = mybir.AluOpType.add

    class _TC(tile.TileContext):
        # This walrus build rejects >1 sync-wait on a TPB_CTRL Drain; split
        # the kernel-tail drain's global-clock waits across several drains.
        def _drain_and_barrier(self, tick_clock, wait_clock):
            drain_inst = self.nc.sync.drain()
            wait_clock.add_sem_waits(
                drain_inst.ins, ScopedClock({None: tick_clock.global_clock})
            )
            si = drain_inst.ins.sync_info
            waits = list(si.on_wait or []) if si is not None else []
            if len(waits) > 1:
                si.on_wait = waits[:1]
                for w in waits[1:]:
                    extra = self.nc.sync.drain()
                    esi = extra.ins.sync_info
                    if esi is None:
                        extra.ins.sync_info = type(si)(on_wait=[w], on_update=[])
                    else:
                        esi.on_wait = [w]
            self.nc.all_engine_barrier()
            assert self.sems is not None
            popped = self.nc._tile_sem_poison_stack.pop()
            assert popped is self._sem_poison
            self.nc.clear_and_free_semaphores(list(self.sems.allocated().values()))
            self.nc.all_engine_barrier()

    def _split_excess_waits(limit=1):
        # Same walrus limitation for regular engine instructions: keep at
        # most `limit` sync-waits per instruction; move the excess onto
        # single-wait NOPs inserted just before it on the same engine
        # (engines execute in order, so semantics are preserved).
        n = 0
        for bb in nc.main_func.blocks:
            new_list = []
            for ins in bb.instructions:
                si = getattr(ins, "sync_info", None)
                waits = list(si.on_wait) if (si is not None and si.on_wait) else []
                eng = getattr(ins, "engine", None)
                if len(waits) > limit and eng is not None and \
                        not isinstance(ins, mybir.InstDrain):
                    extra, keep = waits[:-limit], waits[-limit:]
                    si.on_wait = keep
                    for w in extra:
                        n += 1
                        nop = mybir.InstNoOp(
                            name=f"wsplit-{n}-{ins.name}",
                            sync_info=mybir.SyncInfo(on_wait=[w], on_update=[]),
                            bass_nofuse=True,
                            engine=eng,
                        )
                        nc.register_instruction(nop, overwrite=True)
                        new_list.append(nop)
                new_list.append(ins)
            bb.instructions[:] = new_list

    nc = bass.Bass("TRN2", target_bir_lowering=False, num_devices=NCORES)

    xt = nc.dram_tensor("xt", [C, SHARD_T], bf16, kind="ExternalInput")
    wqk = nc.dram_tensor("wqk", [C, 128], bf16, kind="ExternalInput")
    wv = nc.dram_tensor("wv", [C, H], bf16, kind="ExternalInput")
    diag = nc.dram_tensor("diag", [128, 512], bf16, kind="ExternalInput")
    csel = nc.dram_tensor("csel", [128, 4], f32, kind="ExternalInput")
    ident = nc.dram_tensor("ident", [128, 128], f32, kind="ExternalInput")
    outd = nc.dram_tensor("out", [SHARD_T, H], f32, kind="ExternalOutput")

    RG = [[0, 1], [2, 3], [4, 5], [6, 7]]
    KTB = 64 * 1024  # bf16 elems of the KT region in one half's pack
    PACK = 2 * KTB  # elems per rank per half (KT [64,1024] + V [1024,64])

    with _TC(nc) as tc:
        with tc.tile_pool(name="const", bufs=1) as constp, \
             tc.tile_pool(name="data", bufs=1) as datap, \
             tc.tile_pool(name="work", bufs=1) as workp, \
             tc.tile_pool(name="psum", bufs=1, space="PSUM") as psum, \
             tc.tile_pool(name="dram", bufs=1, space="DRAM") as dram:

            # ---- constants
            wqk_sb = constp.tile([128, 8, 128], bf16)
            nc.sync.dma_start(wqk_sb[:], wqk[:].rearrange("(cc p) h -> p cc h", p=128))
            wv_sb = constp.tile([128, 8, H], bf16)
            nc.sync.dma_start(wv_sb[:], wv[:].rearrange("(cc p) h -> p cc h", p=128))
            diag_sb = constp.tile([128, 512], bf16)
            nc.sync.dma_start(diag_sb[:], diag[:])
            csel_sb = constp.tile([128, 4], f32)
            nc.sync.dma_start(csel_sb[:], csel[:])
            ident_sb = constp.tile([128, 128], f32)
            nc.sync.dma_start(ident_sb[:], ident[:])

            # ---- x^T shard (8 c-chunks)
            xt_sb = datap.tile([128, 8, SHARD_T], bf16)
            for cc in range(8):
                nc.sync.dma_start(xt_sb[:, cc, :], xt[128 * cc:128 * (cc + 1), :])

            # ---- projection outputs (local)
            qt_sb = datap.tile([64, SHARD_T], bf16)
            ktl_sb = datap.tile([64, SHARD_T], bf16)

            # ---- gathered K/V
            kt_all = datap.tile([64, 4096], bf16)  # 32 tiles of [64,128]
            v_all = datap.tile([128, 32 * 65], bf16)  # 32 tiles of [128,65]
            nc.vector.memset(v_all[:], 1.0)  # ones column at col 64 of each tile

            kv_in = [dram.tile([PACK], bf16, name=f"kv_in{h}") for h in range(2)]
            kv_out = [dram.tile([2 * PACK], bf16, name=f"kv_out{h}") for h in range(2)]

            # ================= phase 1: projections + gathers ================
            for half in range(2):
                for s5r in range(2):
                    s5 = 2 * half + s5r
                    sl = slice(512 * s5, 512 * (s5 + 1))
                    # Q|K: psum[0:64]=Q^T, psum[64:128]=K^T for 512 rows
                    qk_ps = psum.tile([128, 512], f32, tag="big", bufs=4,
                                      name=f"qk_ps{s5}")
                    for cc in range(8):
                        nc.tensor.matmul(qk_ps[:], wqk_sb[:, cc, :],
                                         xt_sb[:, cc, sl],
                                         start=(cc == 0), stop=(cc == 7))
                    nc.vector.tensor_copy(qt_sb[:, sl], qk_ps[0:64, :])
                    nc.vector.tensor_copy(ktl_sb[:, sl], qk_ps[64:128, :])
                    # V natural [s,h] for the same 512 rows (4 s128 blocks)
                    v_ps = psum.tile([128, 256], f32, tag="big", bufs=4,
                                     name=f"v_ps{s5}")
                    for tt in range(4):
                        for cc in range(8):
                            nc.tensor.matmul(
                                v_ps[:, 64 * tt:64 * (tt + 1)],
                                xt_sb[:, cc, 512 * s5 + 128 * tt:512 * s5 + 128 * (tt + 1)],
                                wv_sb[:, cc, :],
                                start=(cc == 0), stop=(cc == 7))
                    vloc = workp.tile([128, 256], bf16, tag="vloc", bufs=2,
                                      name=f"vloc{s5}")
                    nc.vector.tensor_copy(vloc[:], v_ps[:])
                    # pack V -> kv_in[half] rows (s5r*512 + 128*tt + p, h)
                    dst = kv_in[half][KTB + s5r * 512 * 64:KTB + (s5r + 1) * 512 * 64]
                    nc.scalar.dma_start(
                        dst.rearrange("(t p h) -> p t h", t=4, p=128),
                        vloc[:].rearrange("p (t h) -> p t h", t=4))
                # pack KT half -> kv_in[half][0:KTB]
                ksl = slice(1024 * half, 1024 * (half + 1))
                nc.scalar.dma_start(
                    kv_in[half][0:KTB].rearrange("(h s) -> h s", h=64),
                    ktl_sb[:, ksl])
                nc.gpsimd.collective_compute(
                    "AllGather", mybir.AluOpType.bypass, replica_groups=RG,
                    ins=[kv_in[half].opt()], outs=[kv_out[half].opt()])
                # unpack: KT (2 DMAs) + V (2 strided DMAs), rank-part p=0,1
                for p in range(2):
                    base_tp = half * 16 + p * 8
                    src_kt = kv_out[half][p * PACK:p * PACK + KTB]
                    nc.sync.dma_start(
                        kt_all[:, 128 * base_tp:128 * (base_tp + 8)],
                        src_kt.rearrange("(h s) -> h s", h=64))
                    src_v = kv_out[half][p * PACK + KTB:(p + 1) * PACK]
                    # dst tiles base_tp..base_tp+7, 64 cols each, stride 65
                    dst_v = v_all[:, 65 * base_tp:65 * (base_tp + 8)] \
                        .rearrange("p (lt w) -> p lt w", lt=8)[:, :, 0:64]
                    nc.sync.dma_start(
                        dst_v, src_v.rearrange("(lt s h) -> s lt h", lt=8, s=128))

            # ================= phase 2: attention =================
            inv_sqrt_c = 1.0 / float(np.sqrt(C))
            Exp = mybir.ActivationFunctionType.Exp
            for k in range(SLOTS):
                q_rhs = qt_sb[:, QC * k:QC * (k + 1)]
                o_ps = psum.tile([65, 256], f32, tag="o", bufs=2, name=f"o_ps{k}")
                ndt = 2 * k + 2  # double-tiles (=key chunks) incl. padding
                for d in range(ndt):
                    st = psum.tile([128, 512], f32, tag="big", bufs=4,
                                   name=f"st{k}_{d}")
                    for e in range(2):
                        tp = _gpos(2 * d + e)
                        nc.tensor.matmul(st[:, 256 * e:256 * (e + 1)],
                                         kt_all[:, 128 * tp:128 * (tp + 1)],
                                         q_rhs, start=True, stop=True)
                    p_sb = workp.tile([128, 512], bf16, tag="p", bufs=6,
                                      name=f"p{k}_{d}")
                    nc.scalar.activation(p_sb[:], st[:], Exp, bias=0.0,
                                         scale=inv_sqrt_c)
                    if d >= 2 * k:  # last two key chunks: data-driven mask
                        t = d - 2 * k
                        for e in range(2):
                            m = workp.tile([128, 256], bf16, tag="m", bufs=4,
                                           name=f"m{k}_{d}_{e}")
                            nc.vector.tensor_scalar(
                                m[:], diag_sb[:, 256 * e:256 * (e + 1)],
                                csel_sb[:, 2 * t:2 * t + 1],
                                csel_sb[:, 2 * t + 1:2 * t + 2],
                                mult, add)
                            nc.vector.tensor_mul(p_sb[:, 256 * e:256 * (e + 1)],
                                                 p_sb[:, 256 * e:256 * (e + 1)],
                                                 m[:])
                    for e in range(2):
                        tp = _gpos(2 * d + e)
                        nc.tensor.matmul(o_ps[:],
                                         v_all[:, 65 * tp:65 * tp + 65],
                                         p_sb[:, 256 * e:256 * (e + 1)],
                                         start=(d == 0 and e == 0),
                                         stop=(d == ndt - 1 and e == 1))
                # epilogue: transpose O^T -> O, divide by denominator row
                o_sb = workp.tile([65, 256], f32, tag="osb", bufs=2,
                                  name=f"osb{k}")
                nc.vector.tensor_copy(o_sb[:], o_ps[:])
                for e in range(2):
                    tr = psum.tile([128, 65], f32, tag="tr", bufs=2,
                                   name=f"tr{k}_{e}")
                    nc.tensor.matmul(tr[:], o_sb[:, 128 * e:128 * (e + 1)],
                                     ident_sb[0:65, 0:65], is_transpose=True)
                    rec = workp.tile([128, 1], f32, tag="rec", bufs=4,
                                     name=f"rec{k}_{e}")
                    nc.vector.reciprocal(rec[:], tr[:, 64:65])
                    ob = workp.tile([128, 64], f32, tag="ob", bufs=4,
                                    name=f"ob{k}_{e}")
                    nc.vector.tensor_scalar_mul(ob[:], tr[:, 0:64], rec[:])
                    nc.gpsimd.dma_start(
                        outd[QC * k + 128 * e:QC * k + 128 * (e + 1), :], ob[:])
    _split_excess_waits()
    return nc


def _host_inputs(x, Wq, Wk, Wv):
    import ml_dtypes
    bf = ml_dtypes.bfloat16
    x = np.asarray(x, np.float32)
    wqk = np.concatenate([np.asarray(Wq, np.float32),
                          np.asarray(Wk, np.float32)], axis=1).astype(bf)
    wv = np.asarray(Wv, np.float32).astype(bf)
    iq = np.arange(QC)[None, :]
    isb = np.arange(128)[:, None]
    d0 = (isb <= iq).astype(np.float32)
    d128 = (isb + 128 <= iq).astype(np.float32)
    diag = np.concatenate([d0, d128], axis=1).astype(bf)  # [128, 512]
    ident = np.eye(128, dtype=np.float32)
    csel_j = [np.array([1, 0, 0, 0], np.float32),  # j=0: t0=diag, t1=zeros
              np.array([0, 1, 1, 0], np.float32)]  # j=1: t0=ones, t1=diag
    in_maps = []
    for core in range(NCORES):
        b, j = core // 2, core % 2
        xs = x[b].reshape(16, QC, C)[j::2].reshape(SHARD_T, C)
        xtc = np.ascontiguousarray(xs.T).astype(bf)
        csel = np.ascontiguousarray(
            np.broadcast_to(csel_j[j][None, :], (128, 4))).astype(np.float32)
        in_maps.append({"xt": xtc, "wqk": wqk, "wv": wv, "diag": diag,
                        "csel": csel, "ident": ident})
    return in_maps


def _assemble(results):
    out = np.empty((B, T, H), np.float32)
    for core in range(NCORES):
        b, j = core // 2, core % 2
        oc = np.asarray(results[core]["out"], np.float32)
        out[b].reshape(16, QC, H)[j::2] = oc.reshape(SLOTS, QC, H)
    return out


def _run(in_maps, trace=False, **kw):
    _ensure_import_path()
    from concourse.bass_utils import run_bass_kernel_spmd
    if "nc" not in _cache:
        _cache["nc"] = _build()
    return run_bass_kernel_spmd(_cache["nc"], in_maps,
                                core_ids=list(range(NCORES)), trace=trace, **kw)


def kernel(x, Wq, Wk, Wv):
    res = _run(_host_inputs(x, Wq, Wk, Wv))
    return _assemble(res.results)


# revision 11
# speedup vs baseline: 8456.3147x; 1.1435x over previous
"""nn_Head_63359357550851: single-head causal attention on 8 TRN2 NeuronCores.

x:[4,4096,1024] f32, Wq/Wk/Wv:[1024,64] f32 -> out:[4,4096,64] f32
  out = softmax(causal(x@Wq @ (x@Wk)^T / sqrt(1024))) @ (x@Wv)

Sharding: core c = (batch b=c//2, parity j=c%2). Each batch is split into 16
query chunks of 256; core (b,j) owns chunks {2k+j : k=0..7} (even/odd
interleave -> perfectly balanced causal work after padding each slot k to
2k+2 key chunks). K/V are projected shard-locally and exchanged within the
(b,0)/(b,1) pair via four pairwise AllGathers (one per quarter of the
sequence) so attention on early slots overlaps the gathers of later keys.

Device kernel (SPMD, identical program on all 8 cores; only data differs):
  warmup:  dummy matmuls during the input DMA window keep the PE HAM busy
           so the clock ramps to 2.4 GHz before real work starts.
  phase 1: QT/KT via packed [Wq|Wk] stationary matmuls (N=512); V projected
           transposed (N=512) then PE-transposed to natural [s,h] tiles.
           Per local quarter: pack K/V, pairwise AllGather, unpack.
  phase 2: S^T-layout flash attention: S^T[s,q] = KT_blk^T @ QT, exp on ACT
           over [128,1024] tiles with the 1/sqrt(C) scale folded in (no max
           pass - scores are bounded: std ~0.25), causal masks built
           on-device from two static diagonal tiles + a per-core 4-scalar
           selector, AV accumulated as O^T = Vaug^T @ P^T with a
           ones-column in V producing the softmax denominator for free.
  epilogue: PE-transpose O^T -> O, multiply by reciprocal denominator.
"""

import sys

import numpy as np

B, T, C, H = 4, 4096, 1024, 64
NCORES = 8
QC = 256  # queries per chunk
SLOTS = 8  # chunks per core
SHARD_T = SLOTS * QC  # 2048 rows per core

_BASS_PATHS = ["/opt/trn_rl_repo", "/root/.axon_site/_ro/trn_rl_repo"]

_cache = {}


def _ensure_import_path():
    for p in _BASS_PATHS:
        if p not in sys.path:
            sys.path.append(p)


def _gpos(sb):
    """Global s128-block index -> tile position in the gathered KV buffers.

    Quarter q gathers global 256-chunks {4q..4q+3}; within a quarter the
    layout is [parity-0 part | parity-1 part], each part two chunks."""
    g, e = sb // 2, sb % 2
    return (g // 4) * 8 + (g % 2) * 4 + ((g % 4) // 2) * 2 + e


def _build():
    _ensure_import_path()
    import concourse.bass as bass
    import concourse.mybir as mybir
    import concourse.tile as tile
    from concourse.vector_clock import ScopedClock

    f32 = mybir.dt.float32
    bf16 = mybir.dt.bfloat16
    mult = mybir.AluOpType.mult
    add = mybir.AluOpType.add

    class _TC(tile.TileContext):
        # This walrus build rejects >1 sync-wait per instruction (the TPB
        # ISA has a single wait slot); split the kernel-tail drain's
        # global-clock waits across several drains.
        def _drain_and_barrier(self, tick_clock, wait_clock):
            drain_inst = self.nc.sync.drain()
            wait_clock.add_sem_waits(
                drain_inst.ins, ScopedClock({None: tick_clock.global_clock})
            )
            si = drain_inst.ins.sync_info
            waits = list(si.on_wait or []) if si is not None else []
            if len(waits) > 1:
                si.on_wait = waits[:1]
                for w in waits[1:]:
                    extra = self.nc.sync.drain()
                    esi = extra.ins.sync_info
                    if esi is None:
                        extra.ins.sync_info = type(si)(on_wait=[w], on_update=[])
                    else:
                        esi.on_wait = [w]
            self.nc.all_engine_barrier()
            assert self.sems is not None
            popped = self.nc._tile_sem_poison_stack.pop()
            assert popped is self._sem_poison
            self.nc.clear_and_free_semaphores(list(self.sems.allocated().values()))
            self.nc.all_engine_barrier()

    def _split_excess_waits(limit=1):
        # Same walrus limitation for regular engine instructions: keep at
        # most `limit` sync-waits per instruction; move the excess onto
        # single-wait NOPs inserted just before it on the same engine
        # (engines execute in order, so semantics are preserved).
        n = 0
        for bb in nc.main_func.blocks:
            new_list = []
            for ins in bb.instructions:
                si = getattr(ins, "sync_info", None)
                waits = list(si.on_wait) if (si is not None and si.on_wait) else []
                eng = getattr(ins, "engine", None)
                if len(waits) > limit and eng is not None and \
                        not isinstance(ins, mybir.InstDrain):
                    extra, keep = waits[:-limit], waits[-limit:]
                    si.on_wait = keep
                    for w in extra:
                        n += 1
                        nop = mybir.InstNoOp(
                            name=f"wsplit-{n}-{ins.name}",
                            sync_info=mybir.SyncInfo(on_wait=[w], on_update=[]),
                            bass_nofuse=True,
                            engine=eng,
                        )
                        nc.register_instruction(nop, overwrite=True)
                        new_list.append(nop)
                new_list.append(ins)
            bb.instructions[:] = new_list

    nc = bass.Bass("TRN2", target_bir_lowering=False, num_devices=NCORES)

    xt = nc.dram_tensor("xt", [C, SHARD_T], bf16, kind="ExternalInput")
    wqk = nc.dram_tensor("wqk", [C, 128], bf16, kind="ExternalInput")
    wv = nc.dram_tensor("wv", [C, H], bf16, kind="ExternalInput")
    diag = nc.dram_tensor("diag", [128, 512], bf16, kind="ExternalInput")
    csel = nc.dram_tensor("csel", [128, 4], f32, kind="ExternalInput")
    ident = nc.dram_tensor("ident", [128, 128], f32, kind="ExternalInput")
    identb = nc.dram_tensor("identb", [128, 128], bf16, kind="ExternalInput")
    outd = nc.dram_tensor("out", [SHARD_T, H], f32, kind="ExternalOutput")

    RG = [[0, 1], [2, 3], [4, 5], [6, 7]]
    KTQ = 64 * 512   # bf16 elems of the KT region in one quarter's pack
    PACK = 2 * KTQ   # elems per rank per quarter (KT [64,512] + V [512,64])

    with _TC(nc) as tc:
        with tc.tile_pool(name="const", bufs=1) as constp, \
             tc.tile_pool(name="data", bufs=1) as datap, \
             tc.tile_pool(name="work", bufs=1) as workp, \
             tc.tile_pool(name="psum", bufs=1, space="PSUM") as psum, \
             tc.tile_pool(name="dram", bufs=1, space="DRAM") as dram:

            # ---- constants
            wqk_sb = constp.tile([128, 8, 128], bf16)
            nc.sync.dma_start(wqk_sb[:], wqk[:].rearrange("(cc p) h -> p cc h", p=128))
            wv_sb = constp.tile([128, 8, H], bf16)
            nc.sync.dma_start(wv_sb[:], wv[:].rearrange("(cc p) h -> p cc h", p=128))
            diag_sb = constp.tile([128, 512], bf16)
            nc.sync.dma_start(diag_sb[:], diag[:])
            csel_sb = constp.tile([128, 4], f32)
            nc.sync.dma_start(csel_sb[:], csel[:])
            ident_sb = constp.tile([128, 128], f32)
            nc.sync.dma_start(ident_sb[:], ident[:])
            identb_sb = constp.tile([128, 128], bf16)
            nc.sync.dma_start(identb_sb[:], identb[:])

            # ---- PE warm-up: dense dummy matmuls while xt streams in, so
            # the PE_HAM up-clocks to 2.4 GHz before the projections start.
            for wi in range(24):
                wu = psum.tile([128, 512], f32, tag="big", bufs=3,
                               padded_shape=[128, 1024], name=f"wu{wi}")
                nc.tensor.matmul(wu[:], wqk_sb[:, 0, :],
                                 wqk_sb[:, 0:4, :].rearrange("p a b -> p (a b)"),
                                 start=True, stop=True)

            # ---- x^T shard (8 c-chunks)
            xt_sb = datap.tile([128, 8, SHARD_T], bf16)
            for cc in range(8):
                nc.sync.dma_start(xt_sb[:, cc, :], xt[128 * cc:128 * (cc + 1), :])

            # ---- projection outputs (local)
            qt_sb = datap.tile([64, SHARD_T], bf16)
            ktl_sb = datap.tile([64, SHARD_T], bf16)

            # ---- gathered K/V
            kt_all = datap.tile([64, 4096], bf16)  # 32 tiles of [64,128]
            v_all = datap.tile([128, 32 * 65], bf16)  # 32 tiles of [128,65]
            nc.vector.memset(v_all[:], 1.0)  # ones column at col 64 of each tile

            kv_in = [dram.tile([PACK], bf16, name=f"kv_in{q}") for q in range(4)]
            kv_out = [dram.tile([2 * PACK], bf16, name=f"kv_out{q}") for q in range(4)]

            # ================= phase 1: projections + quarter gathers =======
            for s5 in range(4):  # local s512 block == exchange quarter
                sl = slice(512 * s5, 512 * (s5 + 1))
                # Q|K: psum[0:64]=Q^T, psum[64:128]=K^T for 512 rows
                qk_ps = psum.tile([128, 512], f32, tag="big", bufs=3,
                                  padded_shape=[128, 1024], name=f"qk_ps{s5}")
                for cc in range(8):
                    nc.tensor.matmul(qk_ps[:], wqk_sb[:, cc, :],
                                     xt_sb[:, cc, sl],
                                     start=(cc == 0), stop=(cc == 7))
                nc.vector.tensor_copy(qt_sb[:, sl], qk_ps[0:64, :])
                nc.vector.tensor_copy(ktl_sb[:, sl], qk_ps[64:128, :])
                # V^T for the same 512 rows, then PE-transpose to natural
                vt_ps = psum.tile([64, 512], f32, tag="big", bufs=3,
                                  padded_shape=[128, 1024], name=f"vt_ps{s5}")
                for cc in range(8):
                    nc.tensor.matmul(vt_ps[:], wv_sb[:, cc, :],
                                     xt_sb[:, cc, sl],
                                     start=(cc == 0), stop=(cc == 7))
                vt_sb = workp.tile([64, 512], bf16, tag="vt", bufs=2,
                                   name=f"vt{s5}")
                nc.vector.tensor_copy(vt_sb[:], vt_ps[:])
                vloc = workp.tile([128, 256], bf16, tag="vloc", bufs=2,
                                  name=f"vloc{s5}")
                for tt in range(4):
                    vn_ps = psum.tile([128, 64], bf16, tag="tr", bufs=1,
                                      name=f"vn{s5}_{tt}")
                    nc.tensor.matmul(vn_ps[:],
                                     vt_sb[:, 128 * tt:128 * (tt + 1)],
                                     identb_sb[0:64, 0:64], is_transpose=True)
                    nc.vector.tensor_copy(vloc[:, 64 * tt:64 * (tt + 1)],
                                          vn_ps[:])
                # pack: KT [64,512] then V rows (128*tt+p, h), then gather
                nc.scalar.dma_start(
                    kv_in[s5][0:KTQ].rearrange("(h s) -> h s", h=64),
                    ktl_sb[:, sl])
                nc.scalar.dma_start(
                    kv_in[s5][KTQ:PACK].rearrange("(t p h) -> p t h", t=4, p=128),
                    vloc[:].rearrange("p (t h) -> p t h", t=4))
                nc.gpsimd.collective_compute(
                    "AllGather", mybir.AluOpType.bypass, replica_groups=RG,
                    ins=[kv_in[s5].opt()], outs=[kv_out[s5].opt()])
                for p in range(2):
                    base_tp = s5 * 8 + p * 4
                    src_kt = kv_out[s5][p * PACK:p * PACK + KTQ]
                    nc.sync.dma_start(
                        kt_all[:, 128 * base_tp:128 * (base_tp + 4)],
                        src_kt.rearrange("(h s) -> h s", h=64))
                    src_v = kv_out[s5][p * PACK + KTQ:(p + 1) * PACK]
                    dst_v = v_all[:, 65 * base_tp:65 * (base_tp + 4)] \
                        .rearrange("p (lt w) -> p lt w", lt=4)[:, :, 0:64]
                    nc.sync.dma_start(
                        dst_v, src_v.rearrange("(lt s h) -> s lt h", lt=4, s=128))

            # ================= phase 2: attention =================
            inv_sqrt_c = 1.0 / float(np.sqrt(C))
            Exp = mybir.ActivationFunctionType.Exp
            for k in range(SLOTS):
                q_rhs = qt_sb[:, QC * k:QC * (k + 1)]
                o_ps = psum.tile([65, 256], f32, tag="o", bufs=1, name=f"o_ps{k}")
                for dp in range(k + 1):  # key-chunk pairs (2dp, 2dp+1)
                    st = psum.tile([128, 1024], f32, tag="big", bufs=3,
                                   name=f"st{k}_{dp}")
                    for e4 in range(4):
                        tp = _gpos(4 * dp + e4)
                        nc.tensor.matmul(st[:, 256 * e4:256 * (e4 + 1)],
                                         kt_all[:, 128 * tp:128 * (tp + 1)],
                                         q_rhs, start=True, stop=True)
                    p_sb = workp.tile([128, 1024], bf16, tag="p", bufs=4,
                                      name=f"p{k}_{dp}")
                    nc.scalar.activation(p_sb[:], st[:], Exp, bias=0.0,
                                         scale=inv_sqrt_c)
                    if dp == k:  # last two key chunks: data-driven mask
                        for e4 in range(4):
                            t, off = e4 // 2, e4 % 2
                            m = workp.tile([128, 256], bf16, tag="m", bufs=4,
                                           name=f"m{k}_{e4}")
                            nc.vector.tensor_scalar(
                                m[:], diag_sb[:, 256 * off:256 * (off + 1)],
                                csel_sb[:, 2 * t:2 * t + 1],
                                csel_sb[:, 2 * t + 1:2 * t + 2],
                                mult, add)
                            nc.vector.tensor_mul(p_sb[:, 256 * e4:256 * (e4 + 1)],
                                                 p_sb[:, 256 * e4:256 * (e4 + 1)],
                                                 m[:])
                    for e4 in range(4):
                        tp = _gpos(4 * dp + e4)
                        nc.tensor.matmul(o_ps[:],
                                         v_all[:, 65 * tp:65 * tp + 65],
                                         p_sb[:, 256 * e4:256 * (e4 + 1)],
                                         start=(dp == 0 and e4 == 0),
                                         stop=(dp == k and e4 == 3))
                # epilogue: transpose O^T -> O, divide by denominator row
                o_sb = workp.tile([65, 256], f32, tag="osb", bufs=2,
                                  name=f"osb{k}")
                nc.vector.tensor_copy(o_sb[:], o_ps[:])
                for e in range(2):
                    tr = psum.tile([128, 65], f32, tag="tr", bufs=1,
                                   name=f"tr{k}_{e}")
                    nc.tensor.matmul(tr[:], o_sb[:, 128 * e:128 * (e + 1)],
                                     ident_sb[0:65, 0:65], is_transpose=True)
                    rec = workp.tile([128, 1], f32, tag="rec", bufs=4,
                                     name=f"rec{k}_{e}")
                    nc.vector.reciprocal(rec[:], tr[:, 64:65])
                    ob = workp.tile([128, 64], f32, tag="ob", bufs=4,
                                    name=f"ob{k}_{e}")
                    nc.vector.tensor_scalar_mul(ob[:], tr[:, 0:64], rec[:])
                    nc.gpsimd.dma_start(
                        outd[QC * k + 128 * e:QC * k + 128 * (e + 1), :], ob[:])
    _split_excess_waits()
    return nc


def _host_inputs(x, Wq, Wk, Wv):
    import ml_dtypes
    bf = ml_dtypes.bfloat16
    x = np.asarray(x, np.float32)
    wqk = np.concatenate([np.asarray(Wq, np.float32),
                          np.asarray(Wk, np.float32)], axis=1).astype(bf)
    wv = np.asarray(Wv, np.float32).astype(bf)
    iq = np.arange(QC)[None, :]
    isb = np.arange(128)[:, None]
    d0 = (isb <= iq).astype(np.float32)
    d128 = (isb + 128 <= iq).astype(np.float32)
    diag = np.concatenate([d0, d128], axis=1).astype(bf)  # [128, 512]
    ident = np.eye(128, dtype=np.float32)
    identb = np.eye(128, dtype=np.float32).astype(bf)
    csel_j = [np.array([1, 0, 0, 0], np.float32),  # j=0: t0=diag, t1=zeros
              np.array([0, 1, 1, 0], np.float32)]  # j=1: t0=ones, t1=diag
    in_maps = []
    for core in range(NCORES):
        b, j = core // 2, core % 2
        xs = x[b].reshape(16, QC, C)[j::2].reshape(SHARD_T, C)
        xtc = np.ascontiguousarray(xs.T).astype(bf)
        csel = np.ascontiguousarray(
            np.broadcast_to(csel_j[j][None, :], (128, 4))).astype(np.float32)
        in_maps.append({"xt": xtc, "wqk": wqk, "wv": wv, "diag": diag,
                        "csel": csel, "ident": ident, "identb": identb})
    return in_maps


def _assemble(results):
    out = np.empty((B, T, H), np.float32)
    for core in range(NCORES):
        b, j = core // 2, core % 2
        oc = np.asarray(results[core]["out"], np.float32)
        out[b].reshape(16, QC, H)[j::2] = oc.reshape(SLOTS, QC, H)
    return out


def _run(in_maps, trace=False, **kw):
    _ensure_import_path()
    from concourse.bass_utils import run_bass_kernel_spmd
    if "nc" not in _cache:
        _cache["nc"] = _build()
    return run_bass_kernel_spmd(_cache["nc"], in_maps,
                                core_ids=list(range(NCORES)), trace=trace, **kw)


def kernel(x, Wq, Wk, Wv):
    res = _run(_host_inputs(x, Wq, Wk, Wv))
    return _assemble(res.results)
